# revision 28
# baseline (speedup 1.0000x reference)
"""Causal self-attention on 8 TRN2 NeuronCores.

Sharding: core_id = 2*b + g  (b = batch 0..3, g = head-group 0..1, 8 heads each).
Each core computes qkv for its 8 heads, causal flash-style attention, and a
partial projection (its 512 channels x full w_proj rows). Host sums the two
partials per batch.

Layout strategy (everything transposed so no on-device transposes are needed):
  - x^T [C, T] per batch (host pre-transposed, bf16)
  - Q^T, K^T computed as W^T @ x^T  -> [512, 2048] (channel on partitions)
  - V computed directly as x @ W_v  -> [2048, 512] (token on partitions),
    stored with a ones column per head (V' = [V_h | 1]) so the attention AV
    matmul also produces the softmax row-sums.
  - S^T = K @ Q^T per 128-token j-chunk, both heads of a pair row-tiled into
    one PSUM tile; exp on ACT; causal zeroing via affine_select on GPSIMD.
  - O^T accumulated in PSUM, normalized with reciprocal+partition_broadcast.
  - proj consumes O^T as the stationary matmul operand.

Scheduling: the attention inner loop is ACT-bound (exp of [128,1024] each
block), so the PE work queue is kept dense by (a) emitting S one block ahead
of the matching AV (the exp->mask->AV chain otherwise head-of-line blocks
the next S), and (b) interleaving QKV/proj matmul units as fillers between
attention blocks via a small work queue with deadline enforcement.
"""

from collections import deque

import numpy as np
import ml_dtypes

import concourse.bass as bass
import concourse.tile as tile
from concourse import bacc, mybir
from concourse.bass_utils import run_bass_kernel_spmd

BF16 = ml_dtypes.bfloat16

B, T, C = 4, 2048, 1024
H = 16               # total heads
D = C // H           # 64
HG = 8               # heads per core (head-group)
CL = HG * D          # 512 local channels
N_CORES = 8
SCALE = 1.0 / float(np.sqrt(D))

NCC = C // 128       # 8 c-chunks
NK = C // 256        # 4 fp8 DoubleRow c-chunks (256 contraction each)
NT4 = T // 512       # 4 t-tiles of 512
NT16 = T // 128      # 16 t-chunks of 128

# qkv inputs are fp8 e4m3 with DoubleRow. w_qkv ~ N(0, 1/C) sits in e4m3's
# denormal range, so the host scales it by WSCALE; Q and K then each carry
# x WSCALE (logits x WSCALE^2, folded into the exp scale) and V carries
# x WSCALE (folded out by setting the V' ones column to WSCALE, so the
# rowsum-normalization divides it away exactly).
WSCALE = 64.0
FP8 = ml_dtypes.float8_e4m3fn

_CACHE = {}


def _emit_body(nc, pools, tensors, use_bias, rep):
    dt = mybir.dt
    psum_s, psum_o, pwork, ywork, norm = pools
    (xt, wqk, wv, xtb, wqkb, wvb, wp, qkt, vps, otp, y_d,
     ones_row, bqk_sb, bv_sb) = tensors

    # ---- small emission units (one PSUM tile each) ----
    # t-tile 0 (tokens 0-511) computes QKV in bf16: the early causal rows
    # average over too few keys to wash out fp8 noise (row 0's output is
    # exactly v_0). Later tiles use fp8 DoubleRow. Both paths carry the
    # uniform x WSCALE on the weights.
    DR = mybir.MatmulPerfMode.DoubleRow

    def emit_qk(mc, tt):
        ps = psum_s.tile([128, 512], dt.float32, tag="s",
                          name=f"mm_qk_{rep}_{mc}_{tt}")
        if tt == 0:
            for cc in range(NCC):
                nc.tensor.matmul(
                    ps[:],
                    wqkb[cc][:, mc * 128:(mc + 1) * 128],
                    xtb[cc][:],
                    start=(cc == 0), stop=(cc == NCC - 1 and not use_bias),
                )
        else:
            for k in range(NK):
                nc.tensor.matmul(
                    ps[:],
                    wqk[k][:, :, mc * 128:(mc + 1) * 128],
                    xt[k][:, :, tt * 512:(tt + 1) * 512],
                    start=(k == 0), stop=(k == NK - 1 and not use_bias),
                    perf_mode=DR,
                )
        if use_bias:
            nc.tensor.matmul(
                ps[:],
                bqk_sb[:, mc * 128:(mc + 1) * 128],
                ones_row[:, tt * 512:(tt + 1) * 512],
                start=False, stop=True,
            )
        nc.vector.tensor_copy(qkt[mc][:, tt * 512:(tt + 1) * 512], ps[:])

    def emit_v(t16):
        ps = psum_s.tile([128, 512], dt.float32, tag="s", name=f"mm_v_{rep}_{t16}")
        if t16 < 4:
            for cc in range(NCC):
                nc.tensor.matmul(
                    ps[:],
                    xtb[cc][:, t16 * 128:(t16 + 1) * 128],
                    wvb[cc][:],
                    start=(cc == 0), stop=(cc == NCC - 1 and not use_bias),
                )
        else:
            for k in range(NK):
                nc.tensor.matmul(
                    ps[:],
                    xt[k][:, :, t16 * 128:(t16 + 1) * 128],
                    wv[k][:],
                    start=(k == 0), stop=(k == NK - 1 and not use_bias),
                    perf_mode=DR,
                )
        if use_bias:
            nc.tensor.matmul(
                ps[:],
                ones_row[:, t16 * 128:(t16 + 1) * 128],
                bv_sb[:],
                start=False, stop=True,
            )
        vt = vps[t16]
        nc.vector.tensor_copy(
            vt[:].rearrange("p (h e) -> p h e", e=D + 1)[:, :, 0:D],
            ps[:].rearrange("p (h d) -> p h d", d=D),
        )

    def emit_proj(qc, nt):
        ps = psum_s.tile([128, 512], dt.float32, tag="s",
                          name=f"mm_y_{rep}_{qc}_{nt}")
        for hp in range(4):
            nc.tensor.matmul(
                ps[:],
                otp[hp][:, qc * 128:(qc + 1) * 128],
                wp[hp][:, nt * 512:(nt + 1) * 512],
                start=(hp == 0), stop=(hp == 3),
            )
        y_sb = ywork.tile([128, 512], dt.bfloat16, tag="y",
                          name=f"y_{rep}_{qc}_{nt}")
        nc.vector.tensor_copy(y_sb[:], ps[:])
        nc.sync.dma_start(
            y_d[qc * 128:(qc + 1) * 128, nt * 512:(nt + 1) * 512],
            y_sb[:])

    # ---- work queue of filler units, with deadline enforcement ----
    # proj units go to a low-priority queue drained only during the last
    # (largest, ACT-bound) sweep so that sweep has PE filler work.
    pending = deque()       # (key, closure)
    pending_late = deque()  # (key, closure)
    emitted = set()

    def push(key, fn):
        pending.append((key, fn))

    def push_late(key, fn):
        pending_late.append((key, fn))

    late_gate = [0]

    def pull(n=1, late=False):
        for _ in range(n):
            q = pending or None
            if q is None and late:
                # rate-limit the late queue so it spreads over the whole
                # last sweep instead of draining in its first blocks
                late_gate[0] ^= 1
                if late_gate[0] and pending_late:
                    q = pending_late
            if not q:
                return
            key, fn = q.popleft()
            if key in emitted:
                continue
            emitted.add(key)
            fn()

    def require(*keys):
        keys = [k for k in keys if k not in emitted]
        if not keys:
            return
        want = set(keys)
        rest = deque()
        while pending:
            key, fn = pending.popleft()
            if key in want and key not in emitted:
                emitted.add(key)
                fn()
                want.discard(key)
            elif key not in emitted:
                rest.append((key, fn))
        pending.extend(rest)
        assert not want, f"missing work units: {want}"

    # ---- attention unit: S one block ahead of AV, fillers between ----
    def emit_attn(hp, qt4):
        q0 = qt4 * 512
        nj = 4 * (qt4 + 1)
        qts, kts = qkt[hp], qkt[4 + hp]
        o_ps = []
        for hi in range(2):
            o_ps.append(psum_o.tile([D + 1, 512], dt.float32, tag="o", bufs=2,
                                    name=f"o_{rep}_{qt4}_{hp}_{hi}"))
        p_tiles = [None] * nj
        cws = [None] * nj

        def emit_s(jc):
            j0 = jc * 128
            off = j0 - q0
            # diagonal blocks: only columns q >= j0 can be unmasked
            c0 = max(0, off)        # first useful column in this q-tile
            w = 512 - c0            # columns computed
            cws[jc] = c0
            s_pair = psum_s.tile([128, 1024], dt.float32, tag="s",
                                 name=f"s_{rep}_{qt4}_{hp}_{jc}")
            for hi in range(2):
                nc.tensor.matmul(
                    s_pair[:, hi * 512 + c0:(hi + 1) * 512],
                    kts[hi * D:(hi + 1) * D, j0:j0 + 128],
                    qts[hi * D:(hi + 1) * D, q0 + c0:q0 + 512],
                    start=True, stop=True,
                )
            p_pair = pwork.tile([128, 1024], dt.bfloat16, tag="p",
                                name=f"p_{rep}_{qt4}_{hp}_{jc}")
            p_tiles[jc] = p_pair
            pv = p_pair[:].rearrange("p (h q) -> p h q", h=2)[:, :, c0:512]
            nc.scalar.activation(
                pv,
                s_pair[:].rearrange("p (h q) -> p h q", h=2)[:, :, c0:512],
                mybir.ActivationFunctionType.Exp,
                scale=SCALE / (WSCALE * WSCALE))
            if off > -128:
                # keep where q_global >= j_global; in the clipped view the
                # column index is qi' = qi - c0, so keep iff qi' >= jj.
                # Only the first 128 columns can violate this (jj <= 127),
                # so mask just that sub-block (4x less gpsimd work).
                mv = p_pair[:].rearrange(
                    "p (h q) -> p h q", h=2)[:, :, c0:c0 + 128]
                nc.gpsimd.affine_select(
                    out=mv, in_=mv,
                    compare_op=mybir.AluOpType.is_ge,
                    fill=0.0, base=0,
                    pattern=[[0, 2], [1, 128]],
                    channel_multiplier=-1,
                )

        def emit_av(jc):
            c0 = cws[jc]
            p_pair = p_tiles[jc]
            for hi in range(2):
                h = 2 * hp + hi
                nc.tensor.matmul(
                    o_ps[hi][:, c0:512],
                    vps[jc][:, h * (D + 1):(h + 1) * (D + 1)],
                    p_pair[:, hi * 512 + c0:(hi + 1) * 512],
                    start=(jc == 0), stop=(jc == nj - 1),
                )

        late = qt4 == NT4 - 1
        emit_s(0)
        for jc in range(1, nj):
            emit_s(jc)
            pull(1, late=late)
            emit_av(jc - 1)
        pull(1, late=late)
        emit_av(nj - 1)

        for hi in range(2):
            recip = norm.tile([1, 512], dt.float32, tag="recip",
                              name=f"recip_{rep}_{qt4}_{hp}_{hi}")
            nc.vector.reciprocal(recip[:], o_ps[hi][D:D + 1, :])
            bcast = norm.tile([D, 512], dt.float32, tag="bcast",
                              name=f"bcast_{rep}_{qt4}_{hp}_{hi}")
            nc.gpsimd.partition_broadcast(bcast[:], recip[:])
            nc.vector.tensor_mul(
                otp[hp][hi * D:(hi + 1) * D, q0:q0 + 512],
                o_ps[hi][0:D, :], bcast[:])

    # ---- build the work schedule ----
    # Lead-in: just what attn(hp=0, qt4=0) needs, so the ACT-bound attention
    # pipeline starts as early as possible.
    emit_qk(0, 0)
    emitted.add(("qk", 0, 0))
    emit_qk(4, 0)
    emitted.add(("qk", 4, 0))
    for t16 in range(4):
        emit_v(t16)
        emitted.add(("v", t16))

    # Fillers for sweep qt4=0: remaining head-pairs' Q/K for t-tile 0.
    for hp in range(1, 4):
        push(("qk", hp, 0), lambda hp=hp: emit_qk(hp, 0))
        push(("qk", 4 + hp, 0), lambda hp=hp: emit_qk(4 + hp, 0))

    for qt4 in range(NT4):
        for hp in range(4):
            require(("qk", hp, qt4), ("qk", 4 + hp, qt4),
                    *[("v", t16) for t16 in range(4 * qt4 + 4)
                      if ("v", t16) not in emitted and t16 >= 4])
            emit_attn(hp, qt4)
            # queue next t-tile's QKV as they become schedulable, plus the
            # previous sweep's proj chunks
            if qt4 < NT4 - 1:
                nt4 = qt4 + 1
                push(("qk", hp, nt4), lambda hp=hp, t=nt4: emit_qk(hp, t))
                push(("qk", 4 + hp, nt4), lambda hp=hp, t=nt4: emit_qk(4 + hp, t))
                if hp < 2:
                    a, b = 4 * qt4 + 4 + 2 * hp, 4 * qt4 + 5 + 2 * hp
                    push(("v", a), lambda t=a: emit_v(t))
                    push(("v", b), lambda t=b: emit_v(t))
            if qt4 > 0:
                # previous sweep's proj: feed into this sweep's filler pulls
                # (each sweep's ACT-vs-PE deficit roughly matches the prior
                # sweep's proj volume)
                qc = 4 * (qt4 - 1) + hp
                push(("proj", qc, 0), lambda qc=qc: emit_proj(qc, 0))
                push(("proj", qc, 1), lambda qc=qc: emit_proj(qc, 1))
    # drain: remaining fillers then last sweep's proj
    while pending or pending_late:
        q = pending if pending else pending_late
        key, fn = q.popleft()
        if key in emitted:
            continue
        emitted.add(key)
        fn()
    for qc in range(4 * (NT4 - 1), 4 * NT4):
        emit_proj(qc, 0)
        emit_proj(qc, 1)


def _build(use_bias: bool, reps: int = 1):
    nc = bacc.Bacc("TRN2", target_bir_lowering=False, debug=False,
                   num_devices=N_CORES)
    dt = mybir.dt

    # fp8 inputs in DoubleRow k-tile layout: row (128k + p) of the dram
    # tensor holds contraction channels c = 256k + 128t + p for t in {0,1},
    # concatenated along the free dim.
    xt_d = nc.dram_tensor("xt", [C // 2, 2 * T], dt.float8e4, kind="ExternalInput").ap()
    wqk_d = nc.dram_tensor("wqk", [C // 2, 2 * 2 * CL], dt.float8e4, kind="ExternalInput").ap()
    wv_d = nc.dram_tensor("wv", [C // 2, 2 * CL], dt.float8e4, kind="ExternalInput").ap()
    xtb_d = nc.dram_tensor("xtb", [C, 512], dt.bfloat16, kind="ExternalInput").ap()
    wqkb_d = nc.dram_tensor("wqkb", [C, 2 * CL], dt.bfloat16, kind="ExternalInput").ap()
    wvb_d = nc.dram_tensor("wvb", [C, CL], dt.bfloat16, kind="ExternalInput").ap()
    wp_d = nc.dram_tensor("wp", [CL, C], dt.bfloat16, kind="ExternalInput").ap()
    bqk_d = bv_d = None
    if use_bias:
        bqk_d = nc.dram_tensor("bqk", [1, 2 * CL], dt.bfloat16, kind="ExternalInput").ap()
        bv_d = nc.dram_tensor("bv", [1, CL], dt.bfloat16, kind="ExternalInput").ap()
    y_d = nc.dram_tensor("y", [T, C], dt.bfloat16, kind="ExternalOutput").ap()

    with tile.TileContext(nc) as tc:
        with (
            tc.tile_pool(name="const", bufs=1) as const,
            tc.tile_pool(name="psum_s", bufs=3, space="PSUM") as psum_s,
            tc.tile_pool(name="psum_o", bufs=2, space="PSUM") as psum_o,
            tc.tile_pool(name="pwork", bufs=8) as pwork,
            tc.tile_pool(name="ywork", bufs=6) as ywork,
            tc.tile_pool(name="norm", bufs=8) as norm,
        ):
            # ---- persistent SBUF inputs ----
            # DMA order: the minimal set for attn(hp0, qt4=0) first (wqk
            # slices for head-pair 0's Q and K, full wv, x^T t-tile 0),
            # then the rest of x^T, then remaining weights.
            xt = [const.tile([128, 2, T], dt.float8e4, tag=f"xt{k}", name=f"xt{k}")
                  for k in range(NK)]
            wqk = [const.tile([128, 2, 2 * CL], dt.float8e4, tag=f"wqk{k}",
                              name=f"wqk{k}") for k in range(NK)]
            wv = [const.tile([128, 2, CL], dt.float8e4, tag=f"wv{k}", name=f"wv{k}")
                  for k in range(NK)]
            xtb = [const.tile([128, 512], dt.bfloat16, tag=f"xtb{cc}",
                              name=f"xtb{cc}") for cc in range(NCC)]
            wqkb = [const.tile([128, 2 * CL], dt.bfloat16, tag=f"wqkb{cc}",
                               name=f"wqkb{cc}") for cc in range(NCC)]
            wvb = [const.tile([128, CL], dt.bfloat16, tag=f"wvb{cc}",
                              name=f"wvb{cc}") for cc in range(NCC)]
            wp = [const.tile([128, C], dt.bfloat16, tag=f"wp{hp}", name=f"wp{hp}")
                  for hp in range(4)]

            def wqk_dv(k):  # dram view of wqk chunk k as [128, 2, 2*CL]
                return wqk_d[k * 128:(k + 1) * 128, :].rearrange(
                    "p (t c) -> p t c", t=2)

            def xt_dv(k):
                return xt_d[k * 128:(k + 1) * 128, :].rearrange(
                    "p (t c) -> p t c", t=2)
            # Progressive order: (1) head-pair 0's bf16 Q/K weight slices +
            # bf16 x^T t-tile 0 (unblocks the first attention unit early),
            # (2) wvb (first AV), (3) remaining bf16 wqkb, (4) fp8 weights +
            # x^T t-tile 1 (sweep-0 fillers), (5) x^T tail, (6) wp (last).
            for cc in range(NCC):
                r = slice(cc * 128, (cc + 1) * 128)
                nc.sync.dma_start(wqkb[cc][:, 0:128], wqkb_d[r, 0:128])
                nc.sync.dma_start(wqkb[cc][:, CL:CL + 128], wqkb_d[r, CL:CL + 128])
                nc.sync.dma_start(xtb[cc][:], xtb_d[r, :])
            for cc in range(NCC):
                r = slice(cc * 128, (cc + 1) * 128)
                nc.sync.dma_start(wvb[cc][:], wvb_d[r, :])
            for cc in range(NCC):
                r = slice(cc * 128, (cc + 1) * 128)
                nc.sync.dma_start(wqkb[cc][:, 128:CL], wqkb_d[r, 128:CL])
                nc.sync.dma_start(wqkb[cc][:, CL + 128:2 * CL],
                                  wqkb_d[r, CL + 128:2 * CL])
            for k in range(NK):
                nc.sync.dma_start(wqk[k][:], wqk_dv(k)[:])
                nc.sync.dma_start(wv[k][:], wv_d[k * 128:(k + 1) * 128, :])
                nc.sync.dma_start(xt[k][:, :, 512:1024], xt_dv(k)[:, :, 512:1024])
            for k in range(NK):
                nc.sync.dma_start(xt[k][:, :, 1024:T], xt_dv(k)[:, :, 1024:T])
            for hp in range(4):
                nc.sync.dma_start(wp[hp][:], wp_d[hp * 128:(hp + 1) * 128, :])
            ones_row = bqk_sb = bv_sb = None
            if use_bias:
                ones_row = const.tile([1, T], dt.bfloat16, tag="ones_row",
                                      name="ones_row")
                nc.vector.memset(ones_row[:], 1.0)
                bqk_sb = const.tile([1, 2 * CL], dt.bfloat16, tag="bqk", name="bqk_sb")
                nc.sync.dma_start(bqk_sb[:], bqk_d[:])
                bv_sb = const.tile([1, CL], dt.bfloat16, tag="bv", name="bv_sb")
                nc.sync.dma_start(bv_sb[:], bv_d[:])

            # persistent intermediate tensors
            qkt = []   # 8 tiles [128, T]: 0..3 = Q^T head-pairs, 4..7 = K^T
            for i in range(8):
                qkt.append(const.tile([128, T], dt.bfloat16, tag=f"qkt{i}",
                                      name=f"qkt{i}"))
            vps = []   # 16 tiles [128, 8*65]: V' per t-chunk
            for i in range(NT16):
                vt = const.tile([128, HG * (D + 1)], dt.bfloat16,
                                tag=f"vp{i}", name=f"vp{i}")
                # "ones" column per head (col 64 of each 65-wide group).
                # Set to WSCALE: V carries a factor WSCALE from the scaled
                # w_v, so the rowsum row carries it too and the recip
                # normalization divides it away exactly.
                nc.vector.memset(
                    vt[:].rearrange("p (h e) -> p h e", e=D + 1)[:, :, D:D + 1],
                    WSCALE)
                vps.append(vt)
            otp = []   # 4 tiles [128, T]: O^T head-pairs
            for hp in range(4):
                otp.append(const.tile([128, T], dt.bfloat16, tag=f"otp{hp}",
                                      name=f"otp{hp}"))

            pools = (psum_s, psum_o, pwork, ywork, norm)
            tensors = (xt, wqk, wv, xtb, wqkb, wvb, wp, qkt, vps, otp, y_d,
                       ones_row, bqk_sb, bv_sb)
            for rep in range(reps):
                _emit_body(nc, pools, tensors, use_bias, rep)

    nc.compile()
    return nc


def _get_nc(use_bias: bool, reps: int = 1):
    key = (use_bias, reps)
    if key not in _CACHE:
        _CACHE[key] = _build(use_bias, reps)
    return _CACHE[key]


def _pack_k(a):
    """[C, F] -> fp8 DoubleRow k-tile layout [C//2, 2*F]: output row
    (128k + p) = concat over t of a[256k + 128t + p, :]."""
    Cr, F = a.shape
    out = a.reshape(Cr // 256, 2, 128, F).transpose(0, 2, 1, 3).reshape(
        Cr // 2, 2 * F)
    return np.ascontiguousarray(out).astype(FP8)


def _make_in_maps(x, w_qkv, b_qkv, w_proj, use_bias):
    xts = [_pack_k(np.ascontiguousarray(x[b].T)) for b in range(B)]
    xtbs = [np.ascontiguousarray(x[b].T[:, 0:512]).astype(BF16) for b in range(B)]
    parts = []
    for g in range(2):
        sl = slice(g * CL, (g + 1) * CL)
        wqk_full = np.concatenate(
            [w_qkv[:, 0:C][:, sl], w_qkv[:, C:2 * C][:, sl]], axis=1) * WSCALE
        wv_full = w_qkv[:, 2 * C:3 * C][:, sl] * WSCALE
        wqk = _pack_k(wqk_full)
        wv = _pack_k(wv_full)
        wp = np.ascontiguousarray(w_proj[sl, :]).astype(BF16)
        d = {"wqk": wqk, "wv": wv, "wp": wp,
             "wqkb": np.ascontiguousarray(wqk_full).astype(BF16),
             "wvb": np.ascontiguousarray(wv_full).astype(BF16)}
        if use_bias:
            d["bqk"] = np.ascontiguousarray(WSCALE * np.concatenate(
                [b_qkv[0:C][sl], b_qkv[C:2 * C][sl]])).astype(BF16).reshape(1, -1)
            d["bv"] = np.ascontiguousarray(
                WSCALE * b_qkv[2 * C:3 * C][sl]).astype(BF16).reshape(1, -1)
        parts.append(d)
    return [dict(parts[core % 2], xt=xts[core // 2], xtb=xtbs[core // 2])
            for core in range(N_CORES)]


def kernel(x, w_qkv, b_qkv, w_proj, b_proj):
    x = np.asarray(x, dtype=np.float32)
    w_qkv = np.asarray(w_qkv, dtype=np.float32)
    b_qkv = np.asarray(b_qkv, dtype=np.float32)
    w_proj = np.asarray(w_proj, dtype=np.float32)
    b_proj = np.asarray(b_proj, dtype=np.float32)

    use_bias = bool(np.any(b_qkv))
    nc = _get_nc(use_bias)
    in_maps = _make_in_maps(x, w_qkv, b_qkv, w_proj, use_bias)

    res = run_bass_kernel_spmd(nc, in_maps, list(range(N_CORES)))
    y = np.empty((B, T, C), dtype=np.float32)
    for b in range(B):
        y[b] = (res.results[2 * b]["y"].astype(np.float32)
                + res.results[2 * b + 1]["y"].astype(np.float32))
    if np.any(b_proj):
        y += b_proj[None, None, :]
    return y


# revision 29
# speedup vs baseline: 4.8792x; 4.8792x over previous
"""Causal self-attention on 8 TRN2 NeuronCores.

Sharding: core_id = 2*b + g  (b = batch 0..3, g = head-group 0..1, 8 heads each).
Each core computes qkv for its 8 heads, causal flash-style attention, and a
partial projection (its 512 channels x full w_proj rows). Host sums the two
partials per batch.

Layout strategy (everything transposed so no on-device transposes are needed):
  - x^T [C, T] per batch (host pre-transposed, bf16)
  - Q^T, K^T computed as W^T @ x^T  -> [512, 2048] (channel on partitions)
  - V computed directly as x @ W_v  -> [2048, 512] (token on partitions),
    stored with a ones column per head (V' = [V_h | 1]) so the attention AV
    matmul also produces the softmax row-sums.
  - S^T = K @ Q^T per 128-token j-chunk, both heads of a pair row-tiled into
    one PSUM tile; exp on ACT; causal zeroing via affine_select on GPSIMD.
  - O^T accumulated in PSUM, normalized with reciprocal+partition_broadcast.
  - proj consumes O^T as the stationary matmul operand.

Scheduling: the attention inner loop is ACT-bound (exp of [128,1024] each
block), so the PE work queue is kept dense by (a) emitting S one block ahead
of the matching AV (the exp->mask->AV chain otherwise head-of-line blocks
the next S), and (b) interleaving QKV/proj matmul units as fillers between
attention blocks via a small work queue with deadline enforcement.
"""

from collections import deque

import numpy as np
import ml_dtypes

import concourse.bass as bass
import concourse.tile as tile
from concourse import bacc, mybir
from concourse.bass_utils import run_bass_kernel_spmd

BF16 = ml_dtypes.bfloat16

B, T, C = 4, 2048, 1024
H = 16               # total heads
D = C // H           # 64
HG = 8               # heads per core (head-group)
CL = HG * D          # 512 local channels
N_CORES = 8
SCALE = 1.0 / float(np.sqrt(D))

NCC = C // 128       # 8 c-chunks
NK = C // 256        # 4 fp8 DoubleRow c-chunks (256 contraction each)
NT4 = T // 512       # 4 t-tiles of 512
NT16 = T // 128      # 16 t-chunks of 128

# qkv inputs are fp8 e4m3 with DoubleRow. w_qkv ~ N(0, 1/C) sits in e4m3's
# denormal range, so the host scales it by WSCALE; Q and K then each carry
# x WSCALE (logits x WSCALE^2, folded into the exp scale) and V carries
# x WSCALE (folded out by setting the V' ones column to WSCALE, so the
# rowsum-normalization divides it away exactly).
WSCALE = 64.0
FP8 = ml_dtypes.float8_e4m3fn

_CACHE = {}


def _emit_body(nc, pools, tensors, use_bias, rep):
    dt = mybir.dt
    psum_s, psum_o, pwork, ywork, norm = pools
    (xt, wqk, wv, xtb, wqkb, wvb, wp, qkt, vps, otp, y_d,
     ones_row, bqk_sb, bv_sb) = tensors

    # ---- small emission units (one PSUM tile each) ----
    # t-tile 0 (tokens 0-511) computes QKV in bf16: the early causal rows
    # average over too few keys to wash out fp8 noise (row 0's output is
    # exactly v_0). Later tiles use fp8 DoubleRow. Both paths carry the
    # uniform x WSCALE on the weights.
    DR = mybir.MatmulPerfMode.DoubleRow

    def emit_qk(mc, tt):
        ps = psum_s.tile([128, 512], dt.float32, tag="s",
                          name=f"mm_qk_{rep}_{mc}_{tt}")
        if tt == 0:
            for cc in range(NCC):
                nc.tensor.matmul(
                    ps[:],
                    wqkb[cc][:, mc * 128:(mc + 1) * 128],
                    xtb[cc][:],
                    start=(cc == 0), stop=(cc == NCC - 1 and not use_bias),
                )
        else:
            for k in range(NK):
                nc.tensor.matmul(
                    ps[:],
                    wqk[k][:, :, mc * 128:(mc + 1) * 128],
                    xt[k][:, :, tt * 512:(tt + 1) * 512],
                    start=(k == 0), stop=(k == NK - 1 and not use_bias),
                    perf_mode=DR,
                )
        if use_bias:
            nc.tensor.matmul(
                ps[:],
                bqk_sb[:, mc * 128:(mc + 1) * 128],
                ones_row[:, tt * 512:(tt + 1) * 512],
                start=False, stop=True,
            )
        nc.vector.tensor_copy(qkt[mc][:, tt * 512:(tt + 1) * 512], ps[:])

    def emit_v(t16):
        ps = psum_s.tile([128, 512], dt.float32, tag="s", name=f"mm_v_{rep}_{t16}")
        if t16 < 4:
            for cc in range(NCC):
                nc.tensor.matmul(
                    ps[:],
                    xtb[cc][:, t16 * 128:(t16 + 1) * 128],
                    wvb[cc][:],
                    start=(cc == 0), stop=(cc == NCC - 1 and not use_bias),
                )
        else:
            for k in range(NK):
                nc.tensor.matmul(
                    ps[:],
                    xt[k][:, :, t16 * 128:(t16 + 1) * 128],
                    wv[k][:],
                    start=(k == 0), stop=(k == NK - 1 and not use_bias),
                    perf_mode=DR,
                )
        if use_bias:
            nc.tensor.matmul(
                ps[:],
                ones_row[:, t16 * 128:(t16 + 1) * 128],
                bv_sb[:],
                start=False, stop=True,
            )
        vt = vps[t16]
        nc.vector.tensor_copy(
            vt[:].rearrange("p (h e) -> p h e", e=D + 1)[:, :, 0:D],
            ps[:].rearrange("p (h d) -> p h d", d=D),
        )

    def emit_proj(qc, nt):
        ps = psum_s.tile([128, 512], dt.float32, tag="s",
                          name=f"mm_y_{rep}_{qc}_{nt}")
        for hp in range(4):
            nc.tensor.matmul(
                ps[:],
                otp[hp][:, qc * 128:(qc + 1) * 128],
                wp[hp][:, nt * 512:(nt + 1) * 512],
                start=(hp == 0), stop=(hp == 3),
            )
        y_sb = ywork.tile([128, 512], dt.bfloat16, tag="y",
                          name=f"y_{rep}_{qc}_{nt}")
        nc.vector.tensor_copy(y_sb[:], ps[:])
        nc.sync.dma_start(
            y_d[qc * 128:(qc + 1) * 128, nt * 512:(nt + 1) * 512],
            y_sb[:])

    # ---- work queue of filler units, with deadline enforcement ----
    # proj units go to a low-priority queue drained only during the last
    # (largest, ACT-bound) sweep so that sweep has PE filler work.
    pending = deque()       # (key, closure)
    pending_late = deque()  # (key, closure)
    emitted = set()

    def push(key, fn):
        pending.append((key, fn))

    def push_late(key, fn):
        pending_late.append((key, fn))

    late_gate = [0]

    def pull(n=1, late=False):
        for _ in range(n):
            q = pending or None
            if q is None and late:
                # rate-limit the late queue so it spreads over the whole
                # last sweep instead of draining in its first blocks
                late_gate[0] ^= 1
                if late_gate[0] and pending_late:
                    q = pending_late
            if not q:
                return
            key, fn = q.popleft()
            if key in emitted:
                continue
            emitted.add(key)
            fn()

    def require(*keys):
        keys = [k for k in keys if k not in emitted]
        if not keys:
            return
        want = set(keys)
        rest = deque()
        while pending:
            key, fn = pending.popleft()
            if key in want and key not in emitted:
                emitted.add(key)
                fn()
                want.discard(key)
            elif key not in emitted:
                rest.append((key, fn))
        pending.extend(rest)
        assert not want, f"missing work units: {want}"

    # ---- attention unit: S one block ahead of AV, fillers between ----
    def emit_attn(hp, qt4):
        q0 = qt4 * 512
        nj = 4 * (qt4 + 1)
        qts, kts = qkt[hp], qkt[4 + hp]
        o_ps = []
        for hi in range(2):
            o_ps.append(psum_o.tile([D + 1, 512], dt.float32, tag="o", bufs=2,
                                    name=f"o_{rep}_{qt4}_{hp}_{hi}"))
        p_tiles = [None] * nj
        cws = [None] * nj

        def emit_s(jc):
            j0 = jc * 128
            off = j0 - q0
            # diagonal blocks: only columns q >= j0 can be unmasked
            c0 = max(0, off)        # first useful column in this q-tile
            w = 512 - c0            # columns computed
            cws[jc] = c0
            s_pair = psum_s.tile([128, 1024], dt.float32, tag="s",
                                 name=f"s_{rep}_{qt4}_{hp}_{jc}")
            for hi in range(2):
                nc.tensor.matmul(
                    s_pair[:, hi * 512 + c0:(hi + 1) * 512],
                    kts[hi * D:(hi + 1) * D, j0:j0 + 128],
                    qts[hi * D:(hi + 1) * D, q0 + c0:q0 + 512],
                    start=True, stop=True,
                )
            p_pair = pwork.tile([128, 1024], dt.bfloat16, tag="p",
                                name=f"p_{rep}_{qt4}_{hp}_{jc}")
            p_tiles[jc] = p_pair
            pv = p_pair[:].rearrange("p (h q) -> p h q", h=2)[:, :, c0:512]
            nc.scalar.activation(
                pv,
                s_pair[:].rearrange("p (h q) -> p h q", h=2)[:, :, c0:512],
                mybir.ActivationFunctionType.Exp,
                scale=SCALE / (WSCALE * WSCALE))
            if off > -128:
                # keep where q_global >= j_global; in the clipped view the
                # column index is qi' = qi - c0, so keep iff qi' >= jj.
                # Only the first 128 columns can violate this (jj <= 127),
                # so mask just that sub-block (4x less gpsimd work).
                mv = p_pair[:].rearrange(
                    "p (h q) -> p h q", h=2)[:, :, c0:c0 + 128]
                nc.gpsimd.affine_select(
                    out=mv, in_=mv,
                    compare_op=mybir.AluOpType.is_ge,
                    fill=0.0, base=0,
                    pattern=[[0, 2], [1, 128]],
                    channel_multiplier=-1,
                )

        def emit_av(jc):
            c0 = cws[jc]
            p_pair = p_tiles[jc]
            for hi in range(2):
                h = 2 * hp + hi
                nc.tensor.matmul(
                    o_ps[hi][:, c0:512],
                    vps[jc][:, h * (D + 1):(h + 1) * (D + 1)],
                    p_pair[:, hi * 512 + c0:(hi + 1) * 512],
                    start=(jc == 0), stop=(jc == nj - 1),
                )

        # S runs 1 block ahead of AV (2 ahead in the diagonal tail, where
        # the exp->mask chain is longest and S blocks are cheap). Fillers
        # are pulled only outside the tail so the psum ring stays free for
        # the deeper pipeline.
        late = qt4 == NT4 - 1
        emit_s(0)
        if nj > 1:
            emit_s(1)
        for jc in range(2, nj):
            emit_s(jc)
            if jc <= nj - 4:
                pull(1, late=late)
            emit_av(jc - 2)
        emit_av(nj - 2)
        emit_av(nj - 1)
        pull(2, late=late)

        for hi in range(2):
            recip = norm.tile([1, 512], dt.float32, tag="recip",
                              name=f"recip_{rep}_{qt4}_{hp}_{hi}")
            nc.vector.reciprocal(recip[:], o_ps[hi][D:D + 1, :])
            bcast = norm.tile([D, 512], dt.float32, tag="bcast",
                              name=f"bcast_{rep}_{qt4}_{hp}_{hi}")
            nc.gpsimd.partition_broadcast(bcast[:], recip[:])
            nc.vector.tensor_mul(
                otp[hp][hi * D:(hi + 1) * D, q0:q0 + 512],
                o_ps[hi][0:D, :], bcast[:])

    # ---- build the work schedule ----
    # Lead-in: just what attn(hp=0, qt4=0) needs, so the ACT-bound attention
    # pipeline starts as early as possible.
    emit_qk(0, 0)
    emitted.add(("qk", 0, 0))
    emit_qk(4, 0)
    emitted.add(("qk", 4, 0))
    for t16 in range(4):
        emit_v(t16)
        emitted.add(("v", t16))

    # Fillers for sweep qt4=0: remaining head-pairs' Q/K for t-tile 0.
    for hp in range(1, 4):
        push(("qk", hp, 0), lambda hp=hp: emit_qk(hp, 0))
        push(("qk", 4 + hp, 0), lambda hp=hp: emit_qk(4 + hp, 0))

    for qt4 in range(NT4):
        for hp in range(4):
            require(("qk", hp, qt4), ("qk", 4 + hp, qt4),
                    *[("v", t16) for t16 in range(4 * qt4 + 4)
                      if ("v", t16) not in emitted and t16 >= 4])
            emit_attn(hp, qt4)
            # queue next t-tile's QKV as they become schedulable, plus the
            # previous sweep's proj chunks
            if qt4 < NT4 - 1:
                nt4 = qt4 + 1
                push(("qk", hp, nt4), lambda hp=hp, t=nt4: emit_qk(hp, t))
                push(("qk", 4 + hp, nt4), lambda hp=hp, t=nt4: emit_qk(4 + hp, t))
                if hp < 2:
                    a, b = 4 * qt4 + 4 + 2 * hp, 4 * qt4 + 5 + 2 * hp
                    push(("v", a), lambda t=a: emit_v(t))
                    push(("v", b), lambda t=b: emit_v(t))
            if qt4 > 0:
                # previous sweep's proj: feed into this sweep's filler pulls
                # (each sweep's ACT-vs-PE deficit roughly matches the prior
                # sweep's proj volume)
                qc = 4 * (qt4 - 1) + hp
                push(("proj", qc, 0), lambda qc=qc: emit_proj(qc, 0))
                push(("proj", qc, 1), lambda qc=qc: emit_proj(qc, 1))
    # drain: remaining fillers then last sweep's proj
    while pending or pending_late:
        q = pending if pending else pending_late
        key, fn = q.popleft()
        if key in emitted:
            continue
        emitted.add(key)
        fn()
    for qc in range(4 * (NT4 - 1), 4 * NT4):
        emit_proj(qc, 0)
        emit_proj(qc, 1)


def _build(use_bias: bool, reps: int = 1):
    nc = bacc.Bacc("TRN2", target_bir_lowering=False, debug=False,
                   num_devices=N_CORES)
    dt = mybir.dt

    # fp8 inputs in DoubleRow k-tile layout: row (128k + p) of the dram
    # tensor holds contraction channels c = 256k + 128t + p for t in {0,1},
    # concatenated along the free dim.
    xt_d = nc.dram_tensor("xt", [C // 2, 2 * T], dt.float8e4, kind="ExternalInput").ap()
    wqk_d = nc.dram_tensor("wqk", [C // 2, 2 * 2 * CL], dt.float8e4, kind="ExternalInput").ap()
    wv_d = nc.dram_tensor("wv", [C // 2, 2 * CL], dt.float8e4, kind="ExternalInput").ap()
    xtb_d = nc.dram_tensor("xtb", [C, 512], dt.bfloat16, kind="ExternalInput").ap()
    wqkb_d = nc.dram_tensor("wqkb", [C, 2 * CL], dt.bfloat16, kind="ExternalInput").ap()
    wvb_d = nc.dram_tensor("wvb", [C, CL], dt.bfloat16, kind="ExternalInput").ap()
    wp_d = nc.dram_tensor("wp", [CL, C], dt.bfloat16, kind="ExternalInput").ap()
    bqk_d = bv_d = None
    if use_bias:
        bqk_d = nc.dram_tensor("bqk", [1, 2 * CL], dt.bfloat16, kind="ExternalInput").ap()
        bv_d = nc.dram_tensor("bv", [1, CL], dt.bfloat16, kind="ExternalInput").ap()
    y_d = nc.dram_tensor("y", [T, C], dt.bfloat16, kind="ExternalOutput").ap()

    with tile.TileContext(nc) as tc:
        with (
            tc.tile_pool(name="const", bufs=1) as const,
            tc.tile_pool(name="psum_s", bufs=3, space="PSUM") as psum_s,
            tc.tile_pool(name="psum_o", bufs=2, space="PSUM") as psum_o,
            tc.tile_pool(name="pwork", bufs=8) as pwork,
            tc.tile_pool(name="ywork", bufs=6) as ywork,
            tc.tile_pool(name="norm", bufs=8) as norm,
        ):
            # ---- persistent SBUF inputs ----
            # DMA order: the minimal set for attn(hp0, qt4=0) first (wqk
            # slices for head-pair 0's Q and K, full wv, x^T t-tile 0),
            # then the rest of x^T, then remaining weights.
            xt = [const.tile([128, 2, T], dt.float8e4, tag=f"xt{k}", name=f"xt{k}")
                  for k in range(NK)]
            wqk = [const.tile([128, 2, 2 * CL], dt.float8e4, tag=f"wqk{k}",
                              name=f"wqk{k}") for k in range(NK)]
            wv = [const.tile([128, 2, CL], dt.float8e4, tag=f"wv{k}", name=f"wv{k}")
                  for k in range(NK)]
            xtb = [const.tile([128, 512], dt.bfloat16, tag=f"xtb{cc}",
                              name=f"xtb{cc}") for cc in range(NCC)]
            wqkb = [const.tile([128, 2 * CL], dt.bfloat16, tag=f"wqkb{cc}",
                               name=f"wqkb{cc}") for cc in range(NCC)]
            wvb = [const.tile([128, CL], dt.bfloat16, tag=f"wvb{cc}",
                              name=f"wvb{cc}") for cc in range(NCC)]
            wp = [const.tile([128, C], dt.bfloat16, tag=f"wp{hp}", name=f"wp{hp}")
                  for hp in range(4)]

            def wqk_dv(k):  # dram view of wqk chunk k as [128, 2, 2*CL]
                return wqk_d[k * 128:(k + 1) * 128, :].rearrange(
                    "p (t c) -> p t c", t=2)

            def xt_dv(k):
                return xt_d[k * 128:(k + 1) * 128, :].rearrange(
                    "p (t c) -> p t c", t=2)
            # Progressive order: (1) head-pair 0's bf16 Q/K weight slices +
            # bf16 x^T t-tile 0 (unblocks the first attention unit early),
            # (2) wvb (first AV), (3) remaining bf16 wqkb, (4) fp8 weights +
            # x^T t-tile 1 (sweep-0 fillers), (5) x^T tail, (6) wp (last).
            for cc in range(NCC):
                r = slice(cc * 128, (cc + 1) * 128)
                nc.sync.dma_start(wqkb[cc][:, 0:128], wqkb_d[r, 0:128])
                nc.sync.dma_start(wqkb[cc][:, CL:CL + 128], wqkb_d[r, CL:CL + 128])
                nc.sync.dma_start(xtb[cc][:], xtb_d[r, :])
            for cc in range(NCC):
                r = slice(cc * 128, (cc + 1) * 128)
                nc.sync.dma_start(wvb[cc][:], wvb_d[r, :])
            for cc in range(NCC):
                r = slice(cc * 128, (cc + 1) * 128)
                nc.sync.dma_start(wqkb[cc][:, 128:CL], wqkb_d[r, 128:CL])
                nc.sync.dma_start(wqkb[cc][:, CL + 128:2 * CL],
                                  wqkb_d[r, CL + 128:2 * CL])
            for k in range(NK):
                nc.sync.dma_start(wqk[k][:], wqk_dv(k)[:])
                nc.sync.dma_start(wv[k][:], wv_d[k * 128:(k + 1) * 128, :])
                nc.sync.dma_start(xt[k][:, :, 512:1024], xt_dv(k)[:, :, 512:1024])
            for k in range(NK):
                nc.sync.dma_start(xt[k][:, :, 1024:T], xt_dv(k)[:, :, 1024:T])
            for hp in range(4):
                nc.sync.dma_start(wp[hp][:], wp_d[hp * 128:(hp + 1) * 128, :])
            ones_row = bqk_sb = bv_sb = None
            if use_bias:
                ones_row = const.tile([1, T], dt.bfloat16, tag="ones_row",
                                      name="ones_row")
                nc.vector.memset(ones_row[:], 1.0)
                bqk_sb = const.tile([1, 2 * CL], dt.bfloat16, tag="bqk", name="bqk_sb")
                nc.sync.dma_start(bqk_sb[:], bqk_d[:])
                bv_sb = const.tile([1, CL], dt.bfloat16, tag="bv", name="bv_sb")
                nc.sync.dma_start(bv_sb[:], bv_d[:])

            # persistent intermediate tensors
            qkt = []   # 8 tiles [128, T]: 0..3 = Q^T head-pairs, 4..7 = K^T
            for i in range(8):
                qkt.append(const.tile([128, T], dt.bfloat16, tag=f"qkt{i}",
                                      name=f"qkt{i}"))
            vps = []   # 16 tiles [128, 8*65]: V' per t-chunk
            for i in range(NT16):
                vt = const.tile([128, HG * (D + 1)], dt.bfloat16,
                                tag=f"vp{i}", name=f"vp{i}")
                # "ones" column per head (col 64 of each 65-wide group).
                # Set to WSCALE: V carries a factor WSCALE from the scaled
                # w_v, so the rowsum row carries it too and the recip
                # normalization divides it away exactly.
                nc.vector.memset(
                    vt[:].rearrange("p (h e) -> p h e", e=D + 1)[:, :, D:D + 1],
                    WSCALE)
                vps.append(vt)
            otp = []   # 4 tiles [128, T]: O^T head-pairs
            for hp in range(4):
                otp.append(const.tile([128, T], dt.bfloat16, tag=f"otp{hp}",
                                      name=f"otp{hp}"))

            pools = (psum_s, psum_o, pwork, ywork, norm)
            tensors = (xt, wqk, wv, xtb, wqkb, wvb, wp, qkt, vps, otp, y_d,
                       ones_row, bqk_sb, bv_sb)
            for rep in range(reps):
                _emit_body(nc, pools, tensors, use_bias, rep)

    nc.compile()
    return nc


def _get_nc(use_bias: bool, reps: int = 1):
    key = (use_bias, reps)
    if key not in _CACHE:
        _CACHE[key] = _build(use_bias, reps)
    return _CACHE[key]


def _pack_k(a):
    """[C, F] -> fp8 DoubleRow k-tile layout [C//2, 2*F]: output row
    (128k + p) = concat over t of a[256k + 128t + p, :]."""
    Cr, F = a.shape
    out = a.reshape(Cr // 256, 2, 128, F).transpose(0, 2, 1, 3).reshape(
        Cr // 2, 2 * F)
    return np.ascontiguousarray(out).astype(FP8)


def _make_in_maps(x, w_qkv, b_qkv, w_proj, use_bias):
    xts = [_pack_k(np.ascontiguousarray(x[b].T)) for b in range(B)]
    xtbs = [np.ascontiguousarray(x[b].T[:, 0:512]).astype(BF16) for b in range(B)]
    parts = []
    for g in range(2):
        sl = slice(g * CL, (g + 1) * CL)
        wqk_full = np.concatenate(
            [w_qkv[:, 0:C][:, sl], w_qkv[:, C:2 * C][:, sl]], axis=1) * WSCALE
        wv_full = w_qkv[:, 2 * C:3 * C][:, sl] * WSCALE
        wqk = _pack_k(wqk_full)
        wv = _pack_k(wv_full)
        wp = np.ascontiguousarray(w_proj[sl, :]).astype(BF16)
        d = {"wqk": wqk, "wv": wv, "wp": wp,
             "wqkb": np.ascontiguousarray(wqk_full).astype(BF16),
             "wvb": np.ascontiguousarray(wv_full).astype(BF16)}
        if use_bias:
            d["bqk"] = np.ascontiguousarray(WSCALE * np.concatenate(
                [b_qkv[0:C][sl], b_qkv[C:2 * C][sl]])).astype(BF16).reshape(1, -1)
            d["bv"] = np.ascontiguousarray(
                WSCALE * b_qkv[2 * C:3 * C][sl]).astype(BF16).reshape(1, -1)
        parts.append(d)
    return [dict(parts[core % 2], xt=xts[core // 2], xtb=xtbs[core // 2])
            for core in range(N_CORES)]


def kernel(x, w_qkv, b_qkv, w_proj, b_proj):
    x = np.asarray(x, dtype=np.float32)
    w_qkv = np.asarray(w_qkv, dtype=np.float32)
    b_qkv = np.asarray(b_qkv, dtype=np.float32)
    w_proj = np.asarray(w_proj, dtype=np.float32)
    b_proj = np.asarray(b_proj, dtype=np.float32)

    use_bias = bool(np.any(b_qkv))
    nc = _get_nc(use_bias)
    in_maps = _make_in_maps(x, w_qkv, b_qkv, w_proj, use_bias)

    res = run_bass_kernel_spmd(nc, in_maps, list(range(N_CORES)))
    y = np.empty((B, T, C), dtype=np.float32)
    for b in range(B):
        y[b] = (res.results[2 * b]["y"].astype(np.float32)
                + res.results[2 * b + 1]["y"].astype(np.float32))
    if np.any(b_proj):
        y += b_proj[None, None, :]
    return y


# revision 32
# speedup vs baseline: 6.1032x; 1.2509x over previous
"""Causal self-attention on 8 TRN2 NeuronCores.

Sharding: core_id = 2*b + g  (b = batch 0..3, g = head-group 0..1, 8 heads each).
Each core computes qkv for its 8 heads, causal flash-style attention, and a
partial projection (its 512 channels x full w_proj rows). Host sums the two
partials per batch.

Layout strategy (everything transposed so no on-device transposes are needed):
  - x^T [C, T] per batch (host pre-transposed, bf16)
  - Q^T, K^T computed as W^T @ x^T  -> [512, 2048] (channel on partitions)
  - V computed directly as x @ W_v  -> [2048, 512] (token on partitions),
    stored with a ones column per head (V' = [V_h | 1]) so the attention AV
    matmul also produces the softmax row-sums.
  - S^T = K @ Q^T per 128-token j-chunk, both heads of a pair row-tiled into
    one PSUM tile; exp on ACT; causal zeroing via affine_select on GPSIMD.
  - O^T accumulated in PSUM, normalized with reciprocal+partition_broadcast.
  - proj consumes O^T as the stationary matmul operand.

Scheduling: the attention inner loop is ACT-bound (exp of [128,1024] each
block), so the PE work queue is kept dense by (a) emitting S one block ahead
of the matching AV (the exp->mask->AV chain otherwise head-of-line blocks
the next S), and (b) interleaving QKV/proj matmul units as fillers between
attention blocks via a small work queue with deadline enforcement.
"""

from collections import deque

import numpy as np
import ml_dtypes

import concourse.bass as bass
import concourse.tile as tile
from concourse import bacc, mybir
from concourse.bass_utils import run_bass_kernel_spmd

BF16 = ml_dtypes.bfloat16

B, T, C = 4, 2048, 1024
H = 16               # total heads
D = C // H           # 64
HG = 8               # heads per core (head-group)
CL = HG * D          # 512 local channels
N_CORES = 8
SCALE = 1.0 / float(np.sqrt(D))

NCC = C // 128       # 8 c-chunks
NK = C // 256        # 4 fp8 DoubleRow c-chunks (256 contraction each)
NT4 = T // 512       # 4 t-tiles of 512
NT16 = T // 128      # 16 t-chunks of 128

# qkv inputs are fp8 e4m3 with DoubleRow. w_qkv ~ N(0, 1/C) sits in e4m3's
# denormal range, so the host scales it by WSCALE; Q and K then each carry
# x WSCALE (logits x WSCALE^2, folded into the exp scale) and V carries
# x WSCALE (folded out by setting the V' ones column to WSCALE, so the
# rowsum-normalization divides it away exactly).
WSCALE = 64.0
FP8 = ml_dtypes.float8_e4m3fn

_CACHE = {}


def _emit_body(nc, pools, tensors, use_bias, rep):
    dt = mybir.dt
    psum_s, psum_o, pwork, ywork, norm = pools
    (xt, wqk, wv, xtb, wqkb, wvb, wp, qkt, vps, otp, y_d,
     ones_row, bqk_sb, bv_sb) = tensors

    # ---- small emission units (one PSUM tile each) ----
    # t-tile 0 (tokens 0-511) computes QKV in bf16: the early causal rows
    # average over too few keys to wash out fp8 noise (row 0's output is
    # exactly v_0). Later tiles use fp8 DoubleRow. Both paths carry the
    # uniform x WSCALE on the weights.
    DR = mybir.MatmulPerfMode.DoubleRow

    def emit_qk(mc, tt):
        ps = psum_s.tile([128, 512], dt.float32, tag="s",
                          name=f"mm_qk_{rep}_{mc}_{tt}")
        if tt == 0:
            for cc in range(NCC):
                nc.tensor.matmul(
                    ps[:],
                    wqkb[cc][:, mc * 128:(mc + 1) * 128],
                    xtb[cc][:],
                    start=(cc == 0), stop=(cc == NCC - 1 and not use_bias),
                )
        else:
            for k in range(NK):
                nc.tensor.matmul(
                    ps[:],
                    wqk[k][:, :, mc * 128:(mc + 1) * 128],
                    xt[k][:, :, tt * 512:(tt + 1) * 512],
                    start=(k == 0), stop=(k == NK - 1 and not use_bias),
                    perf_mode=DR,
                )
        if use_bias:
            nc.tensor.matmul(
                ps[:],
                bqk_sb[:, mc * 128:(mc + 1) * 128],
                ones_row[:, tt * 512:(tt + 1) * 512],
                start=False, stop=True,
            )
        nc.vector.tensor_copy(qkt[mc][:, tt * 512:(tt + 1) * 512], ps[:])

    def emit_v(t16):
        ps = psum_s.tile([128, 512], dt.float32, tag="s", name=f"mm_v_{rep}_{t16}")
        if t16 < 4:
            for cc in range(NCC):
                nc.tensor.matmul(
                    ps[:],
                    xtb[cc][:, t16 * 128:(t16 + 1) * 128],
                    wvb[cc][:],
                    start=(cc == 0), stop=(cc == NCC - 1 and not use_bias),
                )
        else:
            for k in range(NK):
                nc.tensor.matmul(
                    ps[:],
                    xt[k][:, :, t16 * 128:(t16 + 1) * 128],
                    wv[k][:],
                    start=(k == 0), stop=(k == NK - 1 and not use_bias),
                    perf_mode=DR,
                )
        if use_bias:
            nc.tensor.matmul(
                ps[:],
                ones_row[:, t16 * 128:(t16 + 1) * 128],
                bv_sb[:],
                start=False, stop=True,
            )
        vt = vps[t16]
        nc.vector.tensor_copy(
            vt[:].rearrange("p (h e) -> p h e", e=D + 1)[:, :, 0:D],
            ps[:].rearrange("p (h d) -> p h d", d=D),
        )

    def emit_proj(qc, nt):
        ps = psum_s.tile([128, 512], dt.float32, tag="s",
                          name=f"mm_y_{rep}_{qc}_{nt}")
        for hp in range(4):
            nc.tensor.matmul(
                ps[:],
                otp[hp][:, qc * 128:(qc + 1) * 128],
                wp[hp][:, nt * 512:(nt + 1) * 512],
                start=(hp == 0), stop=(hp == 3),
            )
        y_sb = ywork.tile([128, 512], dt.bfloat16, tag="y",
                          name=f"y_{rep}_{qc}_{nt}")
        nc.vector.tensor_copy(y_sb[:], ps[:])
        nc.sync.dma_start(
            y_d[qc * 128:(qc + 1) * 128, nt * 512:(nt + 1) * 512],
            y_sb[:])

    # ---- work queue of filler units, with deadline enforcement ----
    # proj units go to a low-priority queue drained only during the last
    # (largest, ACT-bound) sweep so that sweep has PE filler work.
    pending = deque()       # (key, closure)
    pending_late = deque()  # (key, closure)
    emitted = set()

    def push(key, fn):
        pending.append((key, fn))

    def push_late(key, fn):
        pending_late.append((key, fn))

    late_gate = [0]

    def pull(n=1, late=False):
        for _ in range(n):
            q = pending or None
            if q is None and late:
                # rate-limit the late queue so it spreads over the whole
                # last sweep instead of draining in its first blocks
                late_gate[0] ^= 1
                if late_gate[0] and pending_late:
                    q = pending_late
            if not q:
                return
            key, fn = q.popleft()
            if key in emitted:
                continue
            emitted.add(key)
            fn()

    def require(*keys):
        keys = [k for k in keys if k not in emitted]
        if not keys:
            return
        want = set(keys)
        rest = deque()
        while pending:
            key, fn = pending.popleft()
            if key in want and key not in emitted:
                emitted.add(key)
                fn()
                want.discard(key)
            elif key not in emitted:
                rest.append((key, fn))
        pending.extend(rest)
        assert not want, f"missing work units: {want}"

    # ---- attention unit: S one block ahead of AV, fillers between ----
    def emit_attn(hp, qt4):
        q0 = qt4 * 512
        nj = 4 * (qt4 + 1)
        qts, kts = qkt[hp], qkt[4 + hp]
        o_ps = []
        for hi in range(2):
            o_ps.append(psum_o.tile([D + 1, 512], dt.float32, tag="o", bufs=2,
                                    name=f"o_{rep}_{qt4}_{hp}_{hi}"))
        p_tiles = [None] * nj
        cws = [None] * nj

        def emit_s(jc):
            j0 = jc * 128
            off = j0 - q0
            # diagonal blocks: only columns q >= j0 can be unmasked
            c0 = max(0, off)        # first useful column in this q-tile
            w = 512 - c0            # columns computed
            cws[jc] = c0
            s_pair = psum_s.tile([128, 1024], dt.float32, tag="s",
                                 name=f"s_{rep}_{qt4}_{hp}_{jc}")
            for hi in range(2):
                nc.tensor.matmul(
                    s_pair[:, hi * 512 + c0:(hi + 1) * 512],
                    kts[hi * D:(hi + 1) * D, j0:j0 + 128],
                    qts[hi * D:(hi + 1) * D, q0 + c0:q0 + 512],
                    start=True, stop=True,
                )
            p_pair = pwork.tile([128, 1024], dt.bfloat16, tag="p",
                                name=f"p_{rep}_{qt4}_{hp}_{jc}")
            p_tiles[jc] = p_pair
            pv = p_pair[:].rearrange("p (h q) -> p h q", h=2)[:, :, c0:512]
            nc.scalar.activation(
                pv,
                s_pair[:].rearrange("p (h q) -> p h q", h=2)[:, :, c0:512],
                mybir.ActivationFunctionType.Exp,
                scale=SCALE / (WSCALE * WSCALE))
            if off > -128:
                # keep where q_global >= j_global; in the clipped view the
                # column index is qi' = qi - c0, so keep iff qi' >= jj.
                # Only the first 128 columns can violate this (jj <= 127),
                # so mask just that sub-block (4x less gpsimd work).
                mv = p_pair[:].rearrange(
                    "p (h q) -> p h q", h=2)[:, :, c0:c0 + 128]
                nc.gpsimd.affine_select(
                    out=mv, in_=mv,
                    compare_op=mybir.AluOpType.is_ge,
                    fill=0.0, base=0,
                    pattern=[[0, 2], [1, 128]],
                    channel_multiplier=-1,
                )

        def emit_av(jc):
            c0 = cws[jc]
            p_pair = p_tiles[jc]
            for hi in range(2):
                h = 2 * hp + hi
                nc.tensor.matmul(
                    o_ps[hi][:, c0:512],
                    vps[jc][:, h * (D + 1):(h + 1) * (D + 1)],
                    p_pair[:, hi * 512 + c0:(hi + 1) * 512],
                    start=(jc == 0), stop=(jc == nj - 1),
                )

        # S runs 1 block ahead of AV (2 ahead in the diagonal tail, where
        # the exp->mask chain is longest and S blocks are cheap). Fillers
        # are pulled only outside the tail so the psum ring stays free for
        # the deeper pipeline.
        late = qt4 == NT4 - 1
        emit_s(0)
        if nj > 1:
            emit_s(1)
        for jc in range(2, nj):
            emit_s(jc)
            if jc <= nj - 4 or nj <= 4:
                pull(1, late=late)
            emit_av(jc - 2)
        emit_av(nj - 2)
        emit_av(nj - 1)
        pull(2, late=late)

        for hi in range(2):
            recip = norm.tile([1, 512], dt.float32, tag="recip",
                              name=f"recip_{rep}_{qt4}_{hp}_{hi}")
            nc.vector.reciprocal(recip[:], o_ps[hi][D:D + 1, :])
            bcast = norm.tile([D, 512], dt.float32, tag="bcast",
                              name=f"bcast_{rep}_{qt4}_{hp}_{hi}")
            nc.gpsimd.partition_broadcast(bcast[:], recip[:])
            nc.vector.tensor_mul(
                otp[hp][hi * D:(hi + 1) * D, q0:q0 + 512],
                o_ps[hi][0:D, :], bcast[:])

    # ---- build the work schedule ----
    # Lead-in: just what attn(hp=0, qt4=0) needs, so the ACT-bound attention
    # pipeline starts as early as possible.
    emit_qk(0, 0)
    emitted.add(("qk", 0, 0))
    emit_qk(4, 0)
    emitted.add(("qk", 4, 0))
    for t16 in range(4):
        emit_v(t16)
        emitted.add(("v", t16))

    # Fillers for sweep qt4=0: remaining head-pairs' Q/K for t-tile 0.
    for hp in range(1, 4):
        push(("qk", hp, 0), lambda hp=hp: emit_qk(hp, 0))
        push(("qk", 4 + hp, 0), lambda hp=hp: emit_qk(4 + hp, 0))

    for qt4 in range(NT4):
        for hp in range(4):
            require(("qk", hp, qt4), ("qk", 4 + hp, qt4),
                    *[("v", t16) for t16 in range(4 * qt4 + 4)
                      if ("v", t16) not in emitted and t16 >= 4])
            emit_attn(hp, qt4)
            # queue next t-tile's QKV as they become schedulable, plus the
            # previous sweep's proj chunks
            if qt4 < NT4 - 1:
                nt4 = qt4 + 1
                push(("qk", hp, nt4), lambda hp=hp, t=nt4: emit_qk(hp, t))
                push(("qk", 4 + hp, nt4), lambda hp=hp, t=nt4: emit_qk(4 + hp, t))
                if hp < 2:
                    a, b = 4 * qt4 + 4 + 2 * hp, 4 * qt4 + 5 + 2 * hp
                    push(("v", a), lambda t=a: emit_v(t))
                    push(("v", b), lambda t=b: emit_v(t))
            if qt4 > 0:
                # previous sweep's proj: feed into this sweep's filler pulls
                # (each sweep's ACT-vs-PE deficit roughly matches the prior
                # sweep's proj volume)
                qc = 4 * (qt4 - 1) + hp
                push(("proj", qc, 0), lambda qc=qc: emit_proj(qc, 0))
                push(("proj", qc, 1), lambda qc=qc: emit_proj(qc, 1))
    # drain: remaining fillers then last sweep's proj
    while pending or pending_late:
        q = pending if pending else pending_late
        key, fn = q.popleft()
        if key in emitted:
            continue
        emitted.add(key)
        fn()
    for qc in range(4 * (NT4 - 1), 4 * NT4):
        emit_proj(qc, 0)
        emit_proj(qc, 1)


def _build(use_bias: bool, reps: int = 1):
    nc = bacc.Bacc("TRN2", target_bir_lowering=False, debug=False,
                   num_devices=N_CORES)
    dt = mybir.dt

    # fp8 inputs in DoubleRow k-tile layout: row (128k + p) of the dram
    # tensor holds contraction channels c = 256k + 128t + p for t in {0,1},
    # concatenated along the free dim.
    xt_d = nc.dram_tensor("xt", [C // 2, 2 * T], dt.float8e4, kind="ExternalInput").ap()
    wqk_d = nc.dram_tensor("wqk", [C // 2, 2 * 2 * CL], dt.float8e4, kind="ExternalInput").ap()
    wv_d = nc.dram_tensor("wv", [C // 2, 2 * CL], dt.float8e4, kind="ExternalInput").ap()
    xtb_d = nc.dram_tensor("xtb", [C, 512], dt.bfloat16, kind="ExternalInput").ap()
    wqkb_d = nc.dram_tensor("wqkb", [C, 2 * CL], dt.bfloat16, kind="ExternalInput").ap()
    wvb_d = nc.dram_tensor("wvb", [C, CL], dt.bfloat16, kind="ExternalInput").ap()
    wp_d = nc.dram_tensor("wp", [CL, C], dt.bfloat16, kind="ExternalInput").ap()
    bqk_d = bv_d = None
    if use_bias:
        bqk_d = nc.dram_tensor("bqk", [1, 2 * CL], dt.bfloat16, kind="ExternalInput").ap()
        bv_d = nc.dram_tensor("bv", [1, CL], dt.bfloat16, kind="ExternalInput").ap()
    y_d = nc.dram_tensor("y", [T, C], dt.bfloat16, kind="ExternalOutput").ap()

    with tile.TileContext(nc) as tc:
        with (
            tc.tile_pool(name="const", bufs=1) as const,
            tc.tile_pool(name="psum_s", bufs=3, space="PSUM") as psum_s,
            tc.tile_pool(name="psum_o", bufs=2, space="PSUM") as psum_o,
            tc.tile_pool(name="pwork", bufs=8) as pwork,
            tc.tile_pool(name="ywork", bufs=6) as ywork,
            tc.tile_pool(name="norm", bufs=8) as norm,
        ):
            # ---- persistent SBUF inputs ----
            # DMA order: the minimal set for attn(hp0, qt4=0) first (wqk
            # slices for head-pair 0's Q and K, full wv, x^T t-tile 0),
            # then the rest of x^T, then remaining weights.
            xt = [const.tile([128, 2, T], dt.float8e4, tag=f"xt{k}", name=f"xt{k}")
                  for k in range(NK)]
            wqk = [const.tile([128, 2, 2 * CL], dt.float8e4, tag=f"wqk{k}",
                              name=f"wqk{k}") for k in range(NK)]
            wv = [const.tile([128, 2, CL], dt.float8e4, tag=f"wv{k}", name=f"wv{k}")
                  for k in range(NK)]
            xtb = [const.tile([128, 512], dt.bfloat16, tag=f"xtb{cc}",
                              name=f"xtb{cc}") for cc in range(NCC)]
            wqkb = [const.tile([128, 2 * CL], dt.bfloat16, tag=f"wqkb{cc}",
                               name=f"wqkb{cc}") for cc in range(NCC)]
            wvb = [const.tile([128, CL], dt.bfloat16, tag=f"wvb{cc}",
                              name=f"wvb{cc}") for cc in range(NCC)]
            wp = [const.tile([128, C], dt.bfloat16, tag=f"wp{hp}", name=f"wp{hp}")
                  for hp in range(4)]

            def wqk_dv(k):  # dram view of wqk chunk k as [128, 2, 2*CL]
                return wqk_d[k * 128:(k + 1) * 128, :].rearrange(
                    "p (t c) -> p t c", t=2)

            def xt_dv(k):
                return xt_d[k * 128:(k + 1) * 128, :].rearrange(
                    "p (t c) -> p t c", t=2)
            # Progressive order: (1) head-pair 0's bf16 Q/K weight slices +
            # bf16 x^T t-tile 0 (unblocks the first attention unit early),
            # (2) wvb (first AV), (3) remaining bf16 wqkb, (4) fp8 weights +
            # x^T t-tile 1 (sweep-0 fillers), (5) x^T tail, (6) wp (last).
            def dma(dst, src):
                nc.sync.dma_start(dst, src)

            for cc in range(NCC):
                r = slice(cc * 128, (cc + 1) * 128)
                dma(wqkb[cc][:, 0:128], wqkb_d[r, 0:128])
                dma(wqkb[cc][:, CL:CL + 128], wqkb_d[r, CL:CL + 128])
                dma(xtb[cc][:], xtb_d[r, :])
            for cc in range(NCC):
                r = slice(cc * 128, (cc + 1) * 128)
                dma(wvb[cc][:], wvb_d[r, :])
            for cc in range(NCC):
                r = slice(cc * 128, (cc + 1) * 128)
                dma(wqkb[cc][:, 128:CL], wqkb_d[r, 128:CL])
                dma(wqkb[cc][:, CL + 128:2 * CL], wqkb_d[r, CL + 128:2 * CL])
            for k in range(NK):
                dma(wqk[k][:], wqk_dv(k)[:])
                dma(wv[k][:], wv_d[k * 128:(k + 1) * 128, :])
                dma(xt[k][:, :, 512:1024], xt_dv(k)[:, :, 512:1024])
            for k in range(NK):
                dma(xt[k][:, :, 1024:T], xt_dv(k)[:, :, 1024:T])
            for hp in range(4):
                dma(wp[hp][:], wp_d[hp * 128:(hp + 1) * 128, :])
            ones_row = bqk_sb = bv_sb = None
            if use_bias:
                ones_row = const.tile([1, T], dt.bfloat16, tag="ones_row",
                                      name="ones_row")
                nc.vector.memset(ones_row[:], 1.0)
                bqk_sb = const.tile([1, 2 * CL], dt.bfloat16, tag="bqk", name="bqk_sb")
                nc.sync.dma_start(bqk_sb[:], bqk_d[:])
                bv_sb = const.tile([1, CL], dt.bfloat16, tag="bv", name="bv_sb")
                nc.sync.dma_start(bv_sb[:], bv_d[:])

            # persistent intermediate tensors
            qkt = []   # 8 tiles [128, T]: 0..3 = Q^T head-pairs, 4..7 = K^T
            for i in range(8):
                qkt.append(const.tile([128, T], dt.bfloat16, tag=f"qkt{i}",
                                      name=f"qkt{i}"))
            vps = []   # 16 tiles [128, 8*65]: V' per t-chunk
            for i in range(NT16):
                vt = const.tile([128, HG * (D + 1)], dt.bfloat16,
                                tag=f"vp{i}", name=f"vp{i}")
                # "ones" column per head (col 64 of each 65-wide group).
                # Set to WSCALE: V carries a factor WSCALE from the scaled
                # w_v, so the rowsum row carries it too and the recip
                # normalization divides it away exactly.
                nc.vector.memset(
                    vt[:].rearrange("p (h e) -> p h e", e=D + 1)[:, :, D:D + 1],
                    WSCALE)
                vps.append(vt)
            otp = []   # 4 tiles [128, T]: O^T head-pairs
            for hp in range(4):
                otp.append(const.tile([128, T], dt.bfloat16, tag=f"otp{hp}",
                                      name=f"otp{hp}"))

            pools = (psum_s, psum_o, pwork, ywork, norm)
            tensors = (xt, wqk, wv, xtb, wqkb, wvb, wp, qkt, vps, otp, y_d,
                       ones_row, bqk_sb, bv_sb)
            for rep in range(reps):
                _emit_body(nc, pools, tensors, use_bias, rep)

    nc.compile()
    return nc


def _get_nc(use_bias: bool, reps: int = 1):
    key = (use_bias, reps)
    if key not in _CACHE:
        _CACHE[key] = _build(use_bias, reps)
    return _CACHE[key]


def _pack_k(a):
    """[C, F] -> fp8 DoubleRow k-tile layout [C//2, 2*F]: output row
    (128k + p) = concat over t of a[256k + 128t + p, :]."""
    Cr, F = a.shape
    out = a.reshape(Cr // 256, 2, 128, F).transpose(0, 2, 1, 3).reshape(
        Cr // 2, 2 * F)
    return np.ascontiguousarray(out).astype(FP8)


def _make_in_maps(x, w_qkv, b_qkv, w_proj, use_bias):
    xts = [_pack_k(np.ascontiguousarray(x[b].T)) for b in range(B)]
    xtbs = [np.ascontiguousarray(x[b].T[:, 0:512]).astype(BF16) for b in range(B)]
    parts = []
    for g in range(2):
        sl = slice(g * CL, (g + 1) * CL)
        wqk_full = np.concatenate(
            [w_qkv[:, 0:C][:, sl], w_qkv[:, C:2 * C][:, sl]], axis=1) * WSCALE
        wv_full = w_qkv[:, 2 * C:3 * C][:, sl] * WSCALE
        wqk = _pack_k(wqk_full)
        wv = _pack_k(wv_full)
        wp = np.ascontiguousarray(w_proj[sl, :]).astype(BF16)
        d = {"wqk": wqk, "wv": wv, "wp": wp,
             "wqkb": np.ascontiguousarray(wqk_full).astype(BF16),
             "wvb": np.ascontiguousarray(wv_full).astype(BF16)}
        if use_bias:
            d["bqk"] = np.ascontiguousarray(WSCALE * np.concatenate(
                [b_qkv[0:C][sl], b_qkv[C:2 * C][sl]])).astype(BF16).reshape(1, -1)
            d["bv"] = np.ascontiguousarray(
                WSCALE * b_qkv[2 * C:3 * C][sl]).astype(BF16).reshape(1, -1)
        parts.append(d)
    return [dict(parts[core % 2], xt=xts[core // 2], xtb=xtbs[core // 2])
            for core in range(N_CORES)]


def kernel(x, w_qkv, b_qkv, w_proj, b_proj):
    x = np.asarray(x, dtype=np.float32)
    w_qkv = np.asarray(w_qkv, dtype=np.float32)
    b_qkv = np.asarray(b_qkv, dtype=np.float32)
    w_proj = np.asarray(w_proj, dtype=np.float32)
    b_proj = np.asarray(b_proj, dtype=np.float32)

    use_bias = bool(np.any(b_qkv))
    nc = _get_nc(use_bias)
    in_maps = _make_in_maps(x, w_qkv, b_qkv, w_proj, use_bias)

    res = run_bass_kernel_spmd(nc, in_maps, list(range(N_CORES)))
    y = np.empty((B, T, C), dtype=np.float32)
    for b in range(B):
        y[b] = (res.results[2 * b]["y"].astype(np.float32)
                + res.results[2 * b + 1]["y"].astype(np.float32))
    if np.any(b_proj):
        y += b_proj[None, None, :]
    return y
